# revision 1
# baseline (speedup 1.0000x reference)
"""Causal self-attention (b=4, t=2048, c=1024, 16 heads x 64) on 8 Trainium2
NeuronCores via Bass/Tile.

Sharding: core j -> batch j//2, head-group g=j%2 (8 heads each). Each core
computes qkv for its heads, causal attention, and a partial output projection
(its heads' rows of W_proj); the host sums the two partials per batch and adds
b_proj.

Device kernel design:
- All matmul operands are fp16 (same PE streaming rate as bf16, 8x finer
  mantissa); PSUM accumulation is fp32. Max rel err vs fp32 reference ~4e-4.
- x arrives pre-transposed (xT [c, t]); qT/kT are produced with head-dim on
  partitions (lhsT = W), v with time on partitions (lhsT = xT slices). W_v is
  augmented with a 65th zero column per head whose bias is 1.0, so the AV
  matmul's PSUM row 64 accumulates sum(exp) = the softmax denominator.
- Attention runs on S^T blocks [tk=128, tq=512]: S^T = kT_h.T @ qT_h (row
  tile_position packs even/odd heads into array halves), exp on ScalarE with
  the 1/sqrt(d) scale fused, tk-block-pair-wide (1024) exp ops, AV accumulates
  over tk blocks. Causality: blocks above the diagonal are skipped, diagonal
  blocks compute only the valid tq range plus one [128,128] triangular mask
  multiply on VectorE.
- One-group software pipeline inside each head-pair: the PE queue runs
  S(g+1) before AV(g), so AV's wait on ScalarE's exp(g) is hidden behind the
  next group's S matmuls (pS bufs=2 is exactly enough: pss(g-1) is released
  by its exp before S(g+1) needs a slot).
- Normalization: reciprocal of the PSUM denominator row on VectorE (via an
  SBUF staging copy - the custom-DVE reciprocal must not read PSUM), then a
  64-partition broadcast: DRAM-bounce DMA on the otherwise-idle gpsimd SWDGE
  queue mid-kernel, or a K=1 fp16 ones-matmul for the last chunk so the
  kernel tail never waits on a DMA roundtrip.
- Scheduling: filler thunks (single PE matmuls) drain between attention
  groups to keep TensorE's in-order queue dense (HAM clock gate stays 8/8).
  Two queues: "due" fillers (qkv of chunk ch+1, k/q of late-order pairs
  deferred into the last chunk with drain guards) flush by chunk end; "lazy"
  fillers (projections whose yt slots are never reused) carry across chunks.
  proj(0) runs in ch2; proj(1,2) fill chunk 3's ScalarE-bound attention;
  proj(3) is emitted last and hoisted by the list scheduler via per-pair yt
  tiles. qkv PSUM evacs ride ScalarE (Identity + per-partition bias) to
  unload VectorE; chunk-3-deferred ones stay on VectorE (ScalarE is the
  chunk-3 bottleneck); the final projection evacs use ScalarE (idle by then).
- Startup: ~16 ones-matmuls warm the PE clock gate while input DMAs run on
  two queues (sync HWDGE: x + Wk/Wv/Wq in per-128-row slices so they spread
  across the 16 hardware sub-queues; gpsimd SWDGE: biases, tri mask, Wp).
  Biases arrive pre-shaped [128, 4] (a flat gather would serialize the queue
  with 4-byte descriptors). Output DMAs ride sync (free during the tail).
"""

import numpy as np

N_CORES = 8
B, T, C = 4, 2048, 1024
NH, HD = 16, 64
HPC = 8
HCOLS = HPC * HD  # 512
VAUG = HPC * (HD + 1)  # 520
TCH = 512
NCH = T // TCH
SCALE = 1.0 / np.sqrt(HD)

_CACHE = {}
LAST_RESULTS = None


def _build_nc():
    from concourse import bacc
    import concourse.mybir as mybir
    import concourse.tile as tile

    f32 = mybir.dt.float32
    bf16 = mybir.dt.float16  # fp16: same PE rate as bf16, 8x better mantissa
    fp16 = mybir.dt.float16
    Alu = mybir.AluOpType
    Act = mybir.ActivationFunctionType

    nc = bacc.Bacc("TRN2", target_bir_lowering=False, debug=False, num_devices=N_CORES)

    xT_d = nc.dram_tensor("xT", [C, T], bf16, kind="ExternalInput")
    Wq_d = nc.dram_tensor("Wq", [C, HCOLS], bf16, kind="ExternalInput")
    Wk_d = nc.dram_tensor("Wk", [C, HCOLS], bf16, kind="ExternalInput")
    Wv_d = nc.dram_tensor("Wv", [C, VAUG], bf16, kind="ExternalInput")
    Wp_d = nc.dram_tensor("Wp", [HCOLS, C], bf16, kind="ExternalInput")
    bq_d = nc.dram_tensor("bq", [128, HCOLS // 128], f32, kind="ExternalInput")
    bk_d = nc.dram_tensor("bk", [128, HCOLS // 128], f32, kind="ExternalInput")
    bv_d = nc.dram_tensor("bv", [1, VAUG], f32, kind="ExternalInput")
    tri_d = nc.dram_tensor("TRI", [128, 128], bf16, kind="ExternalInput")
    out_d = nc.dram_tensor("out", [T, C], f32, kind="ExternalOutput")

    KS = C // 128  # 8
    MS = HCOLS // 128  # 4
    TSUB = TCH // 128  # 4

    with tile.TileContext(nc) as tc:
        with (
            tc.tile_pool(name="persist", bufs=1) as persist,
            tc.tile_pool(name="stream", bufs=2) as stream,
            tc.tile_pool(name="es_pool", bufs=10) as esp,
            tc.tile_pool(name="ytp", bufs=3) as ytp,
            tc.tile_pool(name="small", bufs=5) as small,
            tc.tile_pool(name="pA", bufs=2, space="PSUM") as pA,
            tc.tile_pool(name="pS", bufs=2, space="PSUM") as pS,
            tc.tile_pool(name="pY", bufs=2, space="PSUM") as pY,
            tc.tile_pool(name="dram", bufs=8, space="DRAM") as dram,
        ):
            # ---- persistent tiles ----
            bqk = persist.tile([128, 2 * MS], f32, tag="bqk")
            bq = bqk[:, 0:MS]
            bk = bqk[:, MS:2 * MS]
            tri = persist.tile([128, 128], bf16, tag="tri")
            bv_row = persist.tile([1, VAUG], f32, tag="bv_row")
            ones1 = persist.tile([1, 128], f32, tag="ones1")
            onesw = persist.tile([1, 512], fp16, tag="onesw")
            ones1h = persist.tile([1, 64], fp16, tag="ones1h")

            Wk = persist.tile([128, KS, HCOLS], bf16, tag="Wk")
            Wv = persist.tile([128, KS, VAUG], bf16, tag="Wv")
            Wq = persist.tile([128, KS, HCOLS], bf16, tag="Wq")
            Wp = persist.tile([128, MS, C], bf16, tag="Wp")

            kT = persist.tile([128, MS, T], bf16, tag="kT")
            v = persist.tile([128, T // 128, VAUG], bf16, tag="v")
            bvb = persist.tile([128, VAUG], f32, tag="bvb")

            def phase_a_thunks(ch, act_evac=True):
                """(qt_tile, [thunk, ...]) — each thunk emits one PE matmul;
                the last thunk of each 8-matmul group also emits the evac."""
                tsl = slice(ch * TCH, (ch + 1) * TCH)
                xt = stream.tile([128, KS, TCH], bf16, tag="xt")
                xsrc = (xT_d.ap()[:, ch * TCH:(ch + 1) * TCH]
                        .rearrange("(ko p) t -> p ko t", p=128))
                for kk in range(KS):
                    nc.sync.dma_start(xt[:, kk:kk + 1, :], xsrc[:, kk:kk + 1, :])
                qt = stream.tile([128, MS, TCH], bf16, tag="qt")
                thunks = []

                def group(mm_fn, evac_fn, n_mm=KS):
                    st = {}

                    def mk(k):
                        def t():
                            if k == 0:
                                st["ps"] = pA.tile([128, 512], f32, tag="pA",
                                                   name="psa")
                            mm_fn(st["ps"], k)
                            if k == n_mm - 1:
                                evac_fn(st["ps"])
                        return t
                    thunks.extend(mk(k) for k in range(n_mm))

                def k_mm(m, act=act_evac):
                    def mm(ps, k):
                        nc.tensor.matmul(ps[:], Wk[:, k, m * 128:(m + 1) * 128],
                                         xt[:, k, :], start=(k == 0),
                                         stop=(k == KS - 1))

                    def ev(ps):
                        with nc.allow_low_precision(reason="bf16"):
                            if act:
                                nc.scalar.activation(
                                    kT[:, m, tsl], ps[:], Act.Identity,
                                    bias=bk[:, m:m + 1], scale=1.0)
                            else:
                                nc.vector.tensor_scalar(
                                    out=kT[:, m, tsl], in0=ps[:],
                                    scalar1=bk[:, m:m + 1], scalar2=None,
                                    op0=Alu.add)
                    group(mm, ev)

                def v_mm(ts):
                    tk_i = ch * TSUB + ts
                    xsl = xt[:, :, ts * 128:(ts + 1) * 128]

                    def mm(ps, k):
                        nc.tensor.matmul(ps[:], xsl[:, k, :], Wv[:, k, 0:512],
                                         start=(k == 0), stop=(k == KS - 1))

                    def ev(ps):
                        with nc.allow_low_precision(reason="bf16"):
                            nc.vector.tensor_tensor(
                                v[:, tk_i, 0:512], ps[:], bvb[:, 0:512], Alu.add)
                    group(mm, ev)

                    def mm8(ps, k):
                        nc.tensor.matmul(ps[:, 0:8], xsl[:, k, :], Wv[:, k, 512:520],
                                         start=(k == 0), stop=(k == KS - 1))

                    def ev8(ps):
                        with nc.allow_low_precision(reason="bf16"):
                            nc.vector.tensor_tensor(
                                v[:, tk_i, 512:520], ps[:, 0:8], bvb[:, 512:520],
                                Alu.add)
                    group(mm8, ev8)

                def q_mm(m, act=act_evac):
                    def mm(ps, k):
                        nc.tensor.matmul(ps[:], Wq[:, k, m * 128:(m + 1) * 128],
                                         xt[:, k, :], start=(k == 0),
                                         stop=(k == KS - 1))

                    def ev(ps):
                        with nc.allow_low_precision(reason="bf16"):
                            if act:
                                nc.scalar.activation(
                                    qt[:, m, :], ps[:], Act.Identity,
                                    bias=bq[:, m:m + 1], scale=1.0)
                            else:
                                nc.vector.tensor_scalar(
                                    out=qt[:, m, :], in0=ps[:],
                                    scalar1=bq[:, m:m + 1], scalar2=None,
                                    op0=Alu.add)
                    group(mm, ev)

                if ch == NCH - 1:
                    # defer k/q of pairs 1 and 2 (attention order is 3,0,1,2)
                    for m in (3, 0):
                        k_mm(m)
                    for ts in range(TSUB):
                        v_mm(ts)
                    for m in (3, 0):
                        q_mm(m)
                    main = list(thunks)
                    thunks.clear()
                    for m in (1, 2):
                        k_mm(m, act=False)
                        q_mm(m, act=False)
                    deferred = list(thunks)
                    return qt, main, deferred
                for m in range(MS):
                    k_mm(m)
                for ts in range(TSUB):
                    v_mm(ts)
                for m in range(MS):
                    q_mm(m)
                return qt, list(thunks), []

            due_q = []
            lazy_q = []

            drained = [0]  # counts due_q pops (deferred-k/q guard bookkeeping)

            def drain(n):
                while n > 0 and (due_q or lazy_q):
                    if due_q:
                        due_q.pop(0)()
                        drained[0] += 1
                    else:
                        lazy_q.pop(0)()
                    n -= 1

            def emit_pair(ch, qt, yt_p, p, per_drain, mm_bcast=False):
                nblk = (ch + 1) * TSUB
                ngroups = nblk // 2
                hA, hB = 2 * p, 2 * p + 1
                psy = {h: pY.tile([128, 512], f32, tag="pY", name=f"psy{h}")
                       for h in (hA, hB)}

                def emit_S(g):
                    i0, i1 = 2 * g, 2 * g + 1
                    pss = {}
                    # S matmuls: explicit row tile_position for pair concurrency
                    for half, i in ((0, i0), (1, i1)):
                        dk = i - ch * TSUB
                        vs = 128 * dk if dk > 0 else 0
                        for h in (hA, hB):
                            pb = (h % 2) * 64
                            hm = h // 2
                            if half == 0:
                                pss[h] = pS.tile([128, 1024], f32, tag="pS",
                                                 name=f"pss{h}")
                            nc.tensor.matmul(
                                pss[h][:, half * TCH + vs: (half + 1) * TCH],
                                kT[pb:pb + 64, hm, i * 128:(i + 1) * 128],
                                qt[pb:pb + 64, hm, vs:TCH],
                                start=True, stop=True,
                                tile_position=(pb, 0))
                    return pss

                def emit_exp(g, pss):
                    i0, i1 = 2 * g, 2 * g + 1
                    es = {}
                    for h in (hA, hB):
                        es[h] = esp.tile([128, 2 * TCH], bf16, tag="es",
                                         name=f"es{h}")
                        dk1 = i1 - ch * TSUB
                        if dk1 <= 0:
                            with nc.allow_low_precision(reason="bf16"):
                                nc.scalar.activation(es[h][:, :], pss[h][:, :],
                                                     Act.Exp, scale=float(SCALE))
                        else:
                            for half, i in ((0, i0), (1, i1)):
                                dk = i - ch * TSUB
                                vs = 128 * dk if dk > 0 else 0
                                sl = slice(half * TCH + vs, (half + 1) * TCH)
                                with nc.allow_low_precision(reason="bf16"):
                                    nc.scalar.activation(es[h][:, sl], pss[h][:, sl],
                                                         Act.Exp, scale=float(SCALE))
                        for half, i in ((0, i0), (1, i1)):
                            dk = i - ch * TSUB
                            if dk >= 0:
                                vs = 128 * dk
                                sl = slice(half * TCH + vs, half * TCH + vs + 128)
                                with nc.allow_low_precision(reason="bf16"):
                                    nc.vector.tensor_tensor(
                                        es[h][:, sl], es[h][:, sl], tri[:], Alu.mult)
                    return es

                def emit_AV(g, es):
                    i0, i1 = 2 * g, 2 * g + 1
                    for half, i in ((0, i0), (1, i1)):
                        dk = i - ch * TSUB
                        vs = 128 * dk if dk > 0 else 0
                        for h in (hA, hB):
                            nc.tensor.matmul(
                                psy[h][0:65, vs:TCH],
                                v[:, i, h * 65:(h + 1) * 65],
                                es[h][:, half * TCH + vs: (half + 1) * TCH],
                                start=(i == 0), stop=(i == nblk - 1))

                # one-group software pipeline: while ScalarE exponentiates
                # group g, the PE runs group g+1's S matmuls instead of
                # head-blocking on AV(g); pS bufs=2 is exactly enough since
                # pss(g-1) is released by its exp before S(g+1) needs a slot
                es_prev = None
                for g in range(ngroups):
                    pss = emit_S(g)
                    drain(per_drain)
                    es = emit_exp(g, pss)
                    if es_prev is not None:
                        emit_AV(g - 1, es_prev)
                    es_prev = es
                    drain(per_drain)
                emit_AV(ngroups - 1, es_prev)
                # normalization: reciprocal of the denominator row (PSUM row
                # 64), broadcast across 64 partitions per head (DRAM bounce on
                # gpsimd queue, or K=1 fp16 matmul for the kernel tail)
                for h in (hA, hB):
                    pb = (h % 2) * 64
                    lrow = small.tile([1, TCH], f32, tag="lrow", name=f"lrow{h}")
                    nc.vector.tensor_copy(lrow[0:1, :], psy[h][64:65, :])
                    rrow = small.tile([1, TCH], f32, tag="rrow")
                    nc.vector.reciprocal_approx_fast(rrow[0:1, :], lrow[0:1, :])
                    with nc.allow_low_precision(reason="bf16"):
                        # plain evac first so the psum bank frees fast
                        nc.vector.tensor_copy(yt_p[pb:pb + 64, :], psy[h][0:64, :])
                    if mm_bcast:
                        rrowh = small.tile([1, TCH], fp16, tag="rrowh")
                        with nc.allow_low_precision(reason="bf16"):
                            nc.vector.tensor_copy(rrowh[0:1, :], rrow[0:1, :])
                        psb = pA.tile([128, 512], f32, tag="pA", name=f"psb{h}")
                        nc.tensor.matmul(psb[0:64, :], ones1h[0:1, :],
                                         rrowh[0:1, :], start=True, stop=True)
                        with nc.allow_low_precision(reason="bf16"):
                            nc.vector.tensor_tensor(
                                yt_p[pb:pb + 64, :], yt_p[pb:pb + 64, :],
                                psb[0:64, :], Alu.mult)
                    else:
                        drow = dram.tile([1, TCH], f32, tag="drow", name=f"drow{h}")
                        nc.gpsimd.dma_start(drow[:], rrow[:])
                        rbc = small.tile([128, TCH], f32, tag="rbc", name=f"rbc{h}")
                        nc.gpsimd.dma_start(rbc[pb:pb + 64, :],
                                            drow[0:1, :].to_broadcast([64, TCH]))
                        with nc.allow_low_precision(reason="bf16"):
                            nc.vector.tensor_tensor(
                                yt_p[pb:pb + 64, :], yt_p[pb:pb + 64, :],
                                rbc[pb:pb + 64, :], Alu.mult)

            def proj_thunks(ch, yt_map, act_evac=False):
                tq0 = ch * TCH
                thunks = []
                for mt in range(TSUB):
                    for n in range(C // 512):
                        st = {}

                        def mk(k, mt=mt, n=n, st=st):
                            def t():
                                if k == 3:
                                    st["ps"] = pA.tile([128, 512], f32, tag="pA",
                                                       name="psp")
                                nc.tensor.matmul(
                                    st["ps"][:],
                                    yt_map[k][:, mt * 128:(mt + 1) * 128],
                                    Wp[:, k, n * 512:(n + 1) * 512],
                                    start=(k == 3), stop=(k == MS - 2))
                                if k == MS - 2:
                                    ot = small.tile([128, 512], f32, tag="ot")
                                    if act_evac:
                                        nc.scalar.copy(ot[:], st["ps"][:])
                                    else:
                                        nc.vector.tensor_copy(ot[:], st["ps"][:])
                                    nc.sync.dma_start(
                                        out_d.ap()[tq0 + mt * 128:
                                                   tq0 + (mt + 1) * 128,
                                                   n * 512:(n + 1) * 512],
                                        ot[:])
                            return t
                        thunks.extend(mk(k) for k in (3, 0, 1, 2))
                return thunks

            # ---- startup: PE warmup + dual-queue input DMAs ----
            nc.vector.memset(ones1[:], 1.0)
            nc.vector.memset(onesw[:], 1.0)
            with nc.allow_low_precision(reason="exact 1.0"):
                nc.vector.tensor_copy(ones1h[:], ones1[0:1, 0:64])
            # ~16 ones-matmuls keep the PE busy while input DMAs fly so the
            # HAM clock gate reaches 8/8 before the real work starts
            wps = pA.tile([128, 512], f32, tag="pA", name="warm")
            for _ in range(16):
                nc.tensor.matmul(wps[:], onesw[0:1, 0:128], onesw[0:1, :],
                                 start=True, stop=True)
            qt_cur, thunks, _ = phase_a_thunks(0)  # xt(0) DMAs land on sync
            wksrc = Wk_d.ap().rearrange("(ko p) m -> p ko m", p=128)
            wvsrc = Wv_d.ap().rearrange("(ko p) m -> p ko m", p=128)
            wqsrc = Wq_d.ap().rearrange("(ko p) m -> p ko m", p=128)
            for kk in range(KS):
                nc.sync.dma_start(Wk[:, kk:kk + 1, :], wksrc[:, kk:kk + 1, :])
            for kk in range(KS):
                nc.sync.dma_start(Wv[:, kk:kk + 1, :], wvsrc[:, kk:kk + 1, :])
            for kk in range(KS):
                nc.sync.dma_start(Wq[:, kk:kk + 1, :], wqsrc[:, kk:kk + 1, :])
            # gpsimd SWDGE queue: small early tiles + the late-needed Wp
            nc.gpsimd.dma_start(bq, bq_d.ap())
            nc.gpsimd.dma_start(bk, bk_d.ap())
            nc.gpsimd.dma_start(bv_row[:], bv_d.ap())
            nc.gpsimd.dma_start(tri[:], tri_d.ap())
            nc.gpsimd.dma_start(Wp[:], Wp_d.ap().rearrange("(m p) e -> p m e", p=128))

            # bvb: bias broadcast for v evacs
            ps = pA.tile([128, 512], f32, tag="pA")
            nc.tensor.matmul(ps[:, :], ones1[0:1, :], bv_row[0:1, 0:512],
                             start=True, stop=True)
            nc.scalar.copy(bvb[:, 0:512], ps[:, :])
            ps = pA.tile([128, 512], f32, tag="pA")
            nc.tensor.matmul(ps[:, 0:8], ones1[0:1, :], bv_row[0:1, 512:520],
                             start=True, stop=True)
            nc.scalar.copy(bvb[:, 512:520], ps[:, 0:8])

            # ---- main schedule ----
            for t in thunks:
                t()
            yt_all = {}
            defer_v = []
            # proj(c) runs as filler during chunk proj_plan[c]
            proj_plan = {0: 2, 1: 3, 2: 3}
            for ch in range(NCH):
                yt_map = {p: ytp.tile([128, TCH], bf16, tag=f"yt{p}",
                                      name=f"yt{p}_{ch}")
                          for p in (3, 0, 1, 2)}
                yt_all[ch] = yt_map
                if ch + 1 < NCH:
                    qt_next, nxt, defer_kq = phase_a_thunks(ch + 1)
                    due_q.extend(nxt)
                else:
                    qt_next = None
                    drained[0] = 0
                    due_q.extend(defer_kq)
                for c_prev, host in proj_plan.items():
                    if host == ch:
                        # yt(c) slots are reused at chunk c+3: those proj
                        # thunks must flush with the due queue; later ones
                        # (no reuse pending) may carry as gap filler
                        q = lazy_q if c_prev + 3 >= NCH else due_q
                        q.extend(proj_thunks(c_prev, yt_all[c_prev]))
                last = ch == NCH - 1
                pairs = (3, 0, 1, 2)
                # deferred k/q for pair 1 sit in due_q[0:16], pair 2 in
                # [16:32]; they MUST be emitted before that pair's S matmuls
                guards = {1: 16, 2: 32} if last else {}
                for pi, p in enumerate(pairs):
                    if p in guards:
                        drain(max(0, guards[p] - drained[0]))
                    drains_left = 2 * ((ch + 1) * TSUB // 2) * (len(pairs) - pi)
                    pending = len(due_q) + len(lazy_q)
                    per_drain = max(1, (pending + drains_left - 1) // drains_left)
                    emit_pair(ch, qt_cur, yt_map[p], p, per_drain,
                              mm_bcast=(last and p in (1, 2)))
                # qkv fillers for ch+1 must fully land before ch+1 reads
                # qt/kT/v; proj fillers may carry into later chunks
                drain(len(due_q))
                if last:
                    drain(len(lazy_q))
                    for t in proj_thunks(NCH - 1, yt_map, act_evac=True):
                        t()
                qt_cur = qt_next
    nc.compile()
    return nc


def _get_nc():
    if "nc" not in _CACHE:
        _CACHE["nc"] = _build_nc()
    return _CACHE["nc"]


def kernel(x, W_qkv, b_qkv, W_proj, b_proj):
    global LAST_RESULTS
    from concourse.bass_utils import run_bass_kernel_spmd

    x = np.asarray(x, dtype=np.float32)
    W_qkv = np.asarray(W_qkv, dtype=np.float32)
    b_qkv = np.asarray(b_qkv, dtype=np.float32)
    W_proj = np.asarray(W_proj, dtype=np.float32)
    b_proj = np.asarray(b_proj, dtype=np.float32)

    nc = _get_nc()

    tri = np.tril(np.ones((128, 128), dtype=np.float32)).T.copy()  # tri[p,f]=1 iff p<=f

    in_maps = []
    for j in range(N_CORES):
        bi, g = j // 2, j % 2
        c0 = g * HCOLS
        Wv_h = W_qkv[:, 2 * C + c0:2 * C + c0 + HCOLS]
        bv_h = b_qkv[2 * C + c0:2 * C + c0 + HCOLS]
        Wv_aug = np.zeros((C, VAUG), dtype=np.float32)
        bv_aug = np.zeros((1, VAUG), dtype=np.float32)
        for h in range(HPC):
            Wv_aug[:, h * 65:h * 65 + 64] = Wv_h[:, h * 64:(h + 1) * 64]
            bv_aug[0, h * 65:h * 65 + 64] = bv_h[h * 64:(h + 1) * 64]
            bv_aug[0, h * 65 + 64] = 1.0
        bf16 = np.float16
        in_maps.append({
            "xT": x[bi].T.astype(bf16),
            "Wq": W_qkv[:, c0:c0 + HCOLS].astype(bf16),
            "Wk": W_qkv[:, C + c0:C + c0 + HCOLS].astype(bf16),
            "Wv": Wv_aug.astype(bf16),
            "Wp": W_proj[c0:c0 + HCOLS, :].astype(bf16),
            "bq": np.ascontiguousarray(
                b_qkv[c0:c0 + HCOLS].reshape(HCOLS // 128, 128).T),
            "bk": np.ascontiguousarray(
                b_qkv[C + c0:C + c0 + HCOLS].reshape(HCOLS // 128, 128).T),
            "bv": bv_aug,
            "TRI": tri.astype(bf16),
        })

    res = run_bass_kernel_spmd(nc, in_maps, list(range(N_CORES)))
    LAST_RESULTS = res

    out = np.empty((B, T, C), dtype=np.float32)
    for bi in range(B):
        out[bi] = res.results[2 * bi]["out"] + res.results[2 * bi + 1]["out"] + b_proj
    return out

